# revision 2
# baseline (speedup 1.0000x reference)
"""Trainium2 Bass kernel for nn_Activity_Detection: 3-modality bidirectional
LSTM activity head.

Sharding (8 NeuronCores): 4 batch shards (128 rows) x 2 LSTM directions.
Cores 0-3 run the forward LSTMs, cores 4-7 the reverse LSTMs on host
time-reversed features; one SPMD program. Each core computes, per modality,
projT = (x @ W.T + b).T on the PE (audio's 128-wide projection is folded into
its gate weights on the host), then a 20-step LSTM recurrence in bf16 with
fp32 PSUM accumulation, and finally a partial output
(h_a*h_r*h_c) @ W_out_half.T in fp32. The host sums fwd+rev partials + b_out.
"""

import numpy as np
import ml_dtypes

import concourse.bass as bass
import concourse.bacc as bacc
import concourse.tile as tile
import concourse.mybir as mybir
from concourse.masks import make_identity
from concourse import bass_utils

BF16 = mybir.dt.bfloat16
F32 = mybir.dt.float32
AF = mybir.ActivationFunctionType

B, T = 512, 20
RES, C3D, AUD, P, H, NCLS = 2048, 4096, 128, 1024, 1024, 200
BS = 128          # batch rows per core
G4 = 4 * H        # 4096 gate dim
NKH = H // 128    # 8 h chunks
MODS = ("audio", "resnet", "c3d")
DIMS = {"audio": AUD, "resnet": RES, "c3d": C3D}

TRACE = False            # set by test harness for profiling
LAST_RESULTS = None      # BassKernelResults of the last run (for profiling)


def build_program(has_gate_bias: bool):
    nc = bacc.Bacc("TRN2", target_bir_lowering=False, debug=False, num_devices=1)

    x_d = {m: nc.dram_tensor(f"x_{m}", [T, BS, DIMS[m]], BF16, kind="ExternalInput").ap()
           for m in MODS}
    wt_d = {m: nc.dram_tensor(f"wt_{m}", [DIMS[m], P], BF16, kind="ExternalInput").ap()
            for m in ("resnet", "c3d")}
    bp_d = {m: nc.dram_tensor(f"bp_{m}", [P], F32, kind="ExternalInput").ap()
            for m in ("resnet", "c3d")}
    kd = {"audio": AUD + H, "resnet": P + H, "c3d": P + H}
    ww_d = {m: nc.dram_tensor(f"ww_{m}", [kd[m], G4], BF16, kind="ExternalInput").ap()
            for m in MODS}
    gb_d = {m: nc.dram_tensor(f"gb_{m}", [G4], F32, kind="ExternalInput").ap()
            for m in MODS}
    wout_d = nc.dram_tensor("wout", [H, NCLS], F32, kind="ExternalInput").ap()
    out_d = nc.dram_tensor("out_partial", [BS, NCLS], F32, kind="ExternalOutput").ap()
    pjt_d = {m: nc.dram_tensor(f"pjt_{m}", [T, NKH, 128, BS], BF16, kind="Internal").ap()
             for m in ("resnet", "c3d")}

    from contextlib import ExitStack
    with tile.TileContext(nc) as tc, ExitStack() as stack:
        const = stack.enter_context(tc.tile_pool(name="const", bufs=1))
        psum = stack.enter_context(tc.tile_pool(name="psum", bufs=8, space="PSUM"))
        work = stack.enter_context(tc.tile_pool(name="work", bufs=2))
        state = stack.enter_context(tc.tile_pool(name="state", bufs=1))

        ident_bf = const.tile([128, 128], BF16)
        make_identity(nc, ident_bf[:])
        ident_f32 = const.tile([128, 128], F32)
        make_identity(nc, ident_f32[:])

        fused_acc = const.tile([128, H], F32)

        def recurrence(m, st_x_fn, n_kx, wwp):
            """st_x_fn(t, k) -> stationary AP for input-chunk k at step t."""
            n_k = n_kx + NKH
            ww = wwp.tile([128, n_k, G4], BF16, tag="ww")
            nc.sync.dma_start(ww[:], ww_d[m].rearrange("(ko p) n -> p ko n", p=128))
            if has_gate_bias:
                gb_sb = wwp.tile([128, G4], F32, tag="gb")
                nc.sync.dma_start(gb_sb[:], gb_d[m][None, :].to_broadcast([128, G4]))
            hT = state.tile([128, NKH, 128], BF16, tag="hT")
            c_st = state.tile([128, H], F32, tag="c_st")
            h_bf = state.tile([128, H], BF16, tag="h_bf")

            for t in range(T):
                def st(k):
                    return st_x_fn(t, k) if k < n_kx else hT[:, k - n_kx, :]

                ks = list(range(n_k)) if t > 0 else list(range(n_kx))
                G = [psum.tile([128, 512], F32, tag="ps", name=f"g_{m}_{t}_{n}")
                     for n in range(8)]
                for half in (0, 1):
                    for k in ks:
                        for n in range(half * 4, half * 4 + 4):
                            nc.tensor.matmul(
                                G[n][:], st(k), ww[:, k, n * 512:(n + 1) * 512],
                                start=(k == ks[0]), stop=(k == ks[-1]))

                # gate n-chunks: i: G[0:2], f: G[2:4], g: G[4:6], o: G[6:8]
                for j in (0, 1):
                    def gate_in(idx):
                        src = G[idx][:]
                        if has_gate_bias:
                            gs = work.tile([128, 512], F32, tag="gsb")
                            nc.vector.tensor_add(
                                gs[:], src, gb_sb[:, idx * 512:(idx + 1) * 512])
                            src = gs[:]
                        return src

                    sl = slice(j * 512, (j + 1) * 512)
                    sig_f = work.tile([128, 512], F32, tag="sig_f")
                    nc.scalar.activation(sig_f[:], gate_in(2 + j), AF.Sigmoid)
                    if t > 0:
                        nc.vector.tensor_mul(c_st[:, sl], sig_f[:], c_st[:, sl])
                    sig_i = work.tile([128, 512], F32, tag="sig_i")
                    nc.scalar.activation(sig_i[:], gate_in(0 + j), AF.Sigmoid)
                    tanh_g = work.tile([128, 512], F32, tag="tanh_g")
                    nc.scalar.activation(tanh_g[:], gate_in(4 + j), AF.Tanh)
                    if t > 0:
                        tmp2 = work.tile([128, 512], F32, tag="tmp2")
                        nc.vector.tensor_mul(tmp2[:], sig_i[:], tanh_g[:])
                        nc.vector.tensor_add(c_st[:, sl], c_st[:, sl], tmp2[:])
                    else:
                        nc.vector.tensor_mul(c_st[:, sl], sig_i[:], tanh_g[:])
                    tc_t = work.tile([128, 512], F32, tag="tc_t")
                    nc.scalar.activation(tc_t[:], c_st[:, sl], AF.Tanh)
                    sig_o = work.tile([128, 512], F32, tag="sig_o")
                    nc.scalar.activation(sig_o[:], gate_in(6 + j), AF.Sigmoid)
                    if t < T - 1:
                        nc.vector.tensor_mul(h_bf[:, sl], sig_o[:], tc_t[:])
                    else:
                        if m == "audio":
                            nc.vector.tensor_mul(fused_acc[:, sl], sig_o[:], tc_t[:])
                        else:
                            hf = work.tile([128, 512], F32, tag="hf")
                            nc.vector.tensor_mul(hf[:], sig_o[:], tc_t[:])
                            nc.vector.tensor_mul(
                                fused_acc[:, sl], fused_acc[:, sl], hf[:])
                if t < T - 1:
                    for k in range(NKH):
                        tp = psum.tile([128, 512], F32, tag="ps",
                                       name=f"tp_{m}_{t}_{k}")
                        tpv = tp[:, 0:128].bitcast(BF16)[:, 0:128]
                        nc.tensor.transpose(
                            tpv, h_bf[:, k * 128:(k + 1) * 128], ident_bf[:])
                        nc.vector.tensor_copy(hT[:, k, :], tpv)

        # ---------------- audio ----------------
        with tc.tile_pool(name="xta", bufs=1) as xtap, \
             tc.tile_pool(name="wwa", bufs=1) as wwa:
            xt_a = xtap.tile([128, T, 128], BF16)
            for t in range(T):
                nc.sync.dma_start_transpose(xt_a[:, t, :], x_d["audio"][t])
            recurrence("audio", lambda t, k: xt_a[:, t, :], 1, wwa)

        # ------------- resnet / c3d -------------
        RCH = 256  # rows per proj chunk (2 time steps)
        for m in ("resnet", "c3d"):
            dk = DIMS[m] // 128
            with (
                tc.tile_pool(name="wtp", bufs=1) as wtp,
                tc.tile_pool(name="xtp", bufs=2) as xtp,
                tc.tile_pool(name="evp", bufs=4) as evp,
            ):
                bp = const.tile([128, NKH], F32, tag=f"bp_{m}")
                nc.sync.dma_start(bp[:], bp_d[m].rearrange("(mo p) -> p mo", p=128))
                wt = wtp.tile([128, dk, P], BF16, tag="wt")
                nc.sync.dma_start(wt[:], wt_d[m].rearrange("(ko p) n -> p ko n", p=128))
                for r in range(0, T * BS // RCH):
                    xt = xtp.tile([128, dk, RCH], BF16, tag="xt")
                    for tt in range(RCH // BS):
                        t = (r * RCH) // BS + tt
                        nc.sync.dma_start_transpose(
                            xt[:, :, tt * BS:(tt + 1) * BS], x_d[m][t])
                    for half in (0, 1):
                        pp = [psum.tile([128, 512], F32, tag="ps",
                                        name=f"pj_{m}_{r}_{half}_{mm}")
                              for mm in range(4)]
                        for k in range(dk):
                            for mm in range(4):
                                nc.tensor.matmul(
                                    pp[mm][:, 0:RCH],
                                    wt[:, k, (half * 4 + mm) * 128:
                                             (half * 4 + mm + 1) * 128],
                                    xt[:, k, :],
                                    start=(k == 0), stop=(k == dk - 1))
                        for mm in range(4):
                            mo = half * 4 + mm
                            ev = evp.tile([128, RCH], BF16, tag="ev")
                            nc.scalar.activation(
                                ev[:], pp[mm][:, 0:RCH], AF.Identity,
                                bias=bp[:, mo:mo + 1])
                            for tt in range(RCH // BS):
                                t = (r * RCH) // BS + tt
                                nc.sync.dma_start(
                                    pjt_d[m][t, mo],
                                    ev[:, tt * BS:(tt + 1) * BS])

            with tc.tile_pool(name="pjs", bufs=3) as pjs, \
                 tc.tile_pool(name="wwp", bufs=1) as wwp:
                pjt_tiles = {}

                def stream_pjt(t, k, m=m, pjs=pjs, pjt_tiles=pjt_tiles):
                    if t not in pjt_tiles:
                        pt = pjs.tile([128, NKH, BS], BF16, tag="pjt")
                        nc.sync.dma_start(
                            pt[:], pjt_d[m][t].rearrange("mo p b -> p mo b"))
                        pjt_tiles.clear()
                        pjt_tiles[t] = pt
                    return pjt_tiles[t][:, k, :]

                recurrence(m, stream_pjt, NKH, wwp)

        # ---------------- final ----------------
        with tc.tile_pool(name="fin", bufs=1) as fin:
            wo = fin.tile([128, NKH, NCLS], F32)
            nc.sync.dma_start(wo[:], wout_d.rearrange("(ko p) n -> p ko n", p=128))
            fts = []
            for k in range(NKH):
                tp = psum.tile([128, 512], F32, tag="ps", name=f"ft_{k}")
                nc.tensor.transpose(
                    tp[:, 0:128], fused_acc[:, k * 128:(k + 1) * 128], ident_f32[:])
                ft = fin.tile([128, 128], F32, tag=f"ft{k}")
                nc.vector.tensor_copy(ft[:], tp[:, 0:128])
                fts.append(ft)
            ops = psum.tile([128, 512], F32, tag="ps", name="out_ps")
            for k in range(NKH):
                nc.tensor.matmul(ops[:, 0:NCLS], fts[k][:], wo[:, k, :],
                                 start=(k == 0), stop=(k == NKH - 1))
            osb = fin.tile([128, NCLS], F32)
            nc.vector.tensor_copy(osb[:], ops[:, 0:NCLS])
            nc.sync.dma_start(out_d[:], osb[:])

    nc.compile()
    return nc


def _bf16(a):
    return np.ascontiguousarray(a).astype(ml_dtypes.bfloat16)


def host_prep(inputs):
    f = np.float32
    xs = {"audio": inputs["audio_features"], "resnet": inputs["resnet_features"],
          "c3d": inputs["c3d_features"]}
    xt = {m: np.swapaxes(np.asarray(v, f), 0, 1) for m, v in xs.items()}

    wt = {"resnet": _bf16(np.asarray(inputs["W_resnet"], f).T),
          "c3d": _bf16(np.asarray(inputs["W_c3d"], f).T)}
    bp = {"resnet": np.asarray(inputs["b_resnet"], f),
          "c3d": np.asarray(inputs["b_c3d"], f)}

    dirs = {}
    has_gate_bias = False
    for d in ("fwd", "rev"):
        ww = {}
        gb = {}
        for m in MODS:
            wih = np.asarray(inputs[f"{m}_{d}_Wih"], f)
            whh = np.asarray(inputs[f"{m}_{d}_Whh"], f)
            bih = np.asarray(inputs[f"{m}_{d}_bih"], f)
            bhh = np.asarray(inputs[f"{m}_{d}_bhh"], f)
            if m == "audio":
                wa = np.asarray(inputs["W_audio"], f)
                wcomb = wih @ wa                        # (4H, AUD)
                ww[m] = _bf16(np.concatenate([wcomb.T, whh.T], axis=0))
                gb[m] = (wih @ np.asarray(inputs["b_audio"], f) + bih + bhh).astype(f)
            else:
                ww[m] = _bf16(np.concatenate([wih.T, whh.T], axis=0))
                gb[m] = (bih + bhh).astype(f)
            if np.any(gb[m] != 0):
                has_gate_bias = True
        wout_half = (np.asarray(inputs["W_out"], f)[:, :H].T if d == "fwd"
                     else np.asarray(inputs["W_out"], f)[:, H:].T)
        dirs[d] = {"ww": ww, "gb": gb, "wout": np.ascontiguousarray(wout_half)}

    in_maps = []
    for core in range(8):
        d = "fwd" if core < 4 else "rev"
        s = core % 4
        rows = slice(s * BS, (s + 1) * BS)
        im = {}
        for m in MODS:
            xm = xt[m][:, rows]
            if d == "rev":
                xm = xm[::-1]
            im[f"x_{m}"] = _bf16(xm)
            im[f"ww_{m}"] = dirs[d]["ww"][m]
            im[f"gb_{m}"] = dirs[d]["gb"][m]
        for m in ("resnet", "c3d"):
            im[f"wt_{m}"] = wt[m]
            im[f"bp_{m}"] = bp[m]
        im["wout"] = dirs[d]["wout"]
        in_maps.append(im)
    return in_maps, has_gate_bias


def assemble(results, inputs):
    out = np.zeros((B, NCLS), np.float32)
    for s in range(4):
        rows = slice(s * BS, (s + 1) * BS)
        out[rows] = results[s]["out_partial"] + results[4 + s]["out_partial"]
    out += np.asarray(inputs["b_out"], np.float32)[None, :]
    return out


def kernel(**inputs):
    global LAST_RESULTS
    in_maps, has_gate_bias = host_prep(inputs)
    nc = build_program(has_gate_bias)
    res = bass_utils.run_bass_kernel_spmd(
        nc, in_maps, core_ids=list(range(8)), trace=TRACE)
    LAST_RESULTS = res
    return assemble(res.results, inputs)


# revision 6
# speedup vs baseline: 1.1720x; 1.1720x over previous
"""Trainium2 Bass kernel for nn_Activity_Detection: 3-modality bidirectional
LSTM activity head.

Sharding (8 NeuronCores): 4 batch shards (128 rows) x 2 LSTM directions.
Cores 0-3 run the forward LSTMs, cores 4-7 the reverse LSTMs on host
time-reversed features; one SPMD program. Each core computes, per modality,
projT = (x @ W.T + b).T on the PE (audio's 128-wide projection is folded into
its gate weights on the host), then a 20-step LSTM recurrence in bf16 with
fp32 PSUM accumulation, and finally a partial output
(h_a*h_r*h_c) @ W_out_half.T in fp32. The host sums fwd+rev partials + b_out.
"""

import numpy as np
import ml_dtypes

import concourse.bass as bass
import concourse.bacc as bacc
import concourse.tile as tile
import concourse.mybir as mybir
from concourse.masks import make_identity
from concourse import bass_utils

BF16 = mybir.dt.bfloat16
F32 = mybir.dt.float32
AF = mybir.ActivationFunctionType

B, T = 512, 20
RES, C3D, AUD, P, H, NCLS = 2048, 4096, 128, 1024, 1024, 200
BS = 128          # batch rows per core
G4 = 4 * H        # 4096 gate dim
NKH = H // 128    # 8 h chunks
MODS = ("audio", "resnet", "c3d")
DIMS = {"audio": AUD, "resnet": RES, "c3d": C3D}

TRACE = False            # set by test harness for profiling
LAST_RESULTS = None      # BassKernelResults of the last run (for profiling)


def build_program(has_gate_bias: bool):
    nc = bacc.Bacc("TRN2", target_bir_lowering=False, debug=False, num_devices=1)

    x_d = {m: nc.dram_tensor(f"x_{m}", [T, BS, DIMS[m]], BF16, kind="ExternalInput").ap()
           for m in MODS}
    wt_d = {m: nc.dram_tensor(f"wt_{m}", [DIMS[m], P], BF16, kind="ExternalInput").ap()
            for m in ("resnet", "c3d")}
    bp_d = {m: nc.dram_tensor(f"bp_{m}", [P], F32, kind="ExternalInput").ap()
            for m in ("resnet", "c3d")}
    kd = {"audio": AUD + H, "resnet": P + H, "c3d": P + H}
    ww_d = {m: nc.dram_tensor(f"ww_{m}", [kd[m], G4], BF16, kind="ExternalInput").ap()
            for m in MODS}
    gb_d = {m: nc.dram_tensor(f"gb_{m}", [G4], F32, kind="ExternalInput").ap()
            for m in MODS}
    wout_d = nc.dram_tensor("wout", [H, NCLS], F32, kind="ExternalInput").ap()
    out_d = nc.dram_tensor("out_partial", [BS, NCLS], F32, kind="ExternalOutput").ap()
    pjt_d = {m: nc.dram_tensor(f"pjt_{m}", [T, NKH, 128, BS], BF16, kind="Internal").ap()
             for m in ("resnet", "c3d")}

    from contextlib import ExitStack
    with tile.TileContext(nc) as tc, ExitStack() as stack:
        const = stack.enter_context(tc.tile_pool(name="const", bufs=1))
        psum = stack.enter_context(tc.tile_pool(name="psum", bufs=6, space="PSUM"))
        tpsum = stack.enter_context(tc.tile_pool(name="tpsum", bufs=2, space="PSUM"))
        work = stack.enter_context(tc.tile_pool(name="work", bufs=2))
        state = stack.enter_context(tc.tile_pool(name="state", bufs=1))

        ident_bf = const.tile([128, 128], BF16)
        make_identity(nc, ident_bf[:])
        ident_f32 = const.tile([128, 128], F32)
        make_identity(nc, ident_f32[:])

        fused_acc = const.tile([128, H], F32)

        def recurrence(m, st_x_fn, n_kx, wwp):
            """st_x_fn(t, k) -> stationary AP for input-chunk k at step t."""
            n_k = n_kx + NKH
            ww = wwp.tile([128, n_k, G4], BF16, tag="ww")
            wwr = ww_d[m].rearrange("(ko p) n -> p ko n", p=128)
            for k in range(n_k):
                nc.sync.dma_start(ww[:, k], wwr[:, k])
            if has_gate_bias:
                gb_sb = wwp.tile([128, G4], F32, tag="gb")
                nc.sync.dma_start(gb_sb[:], gb_d[m][None, :].to_broadcast([128, G4]))
            hT = state.tile([128, NKH, 128], BF16, tag="hT")
            c_st = state.tile([128, H], F32, tag="c_st")
            h_bf = state.tile([128, H], BF16, tag="h_bf")

            for t in range(T):
                def st(k):
                    return st_x_fn(t, k) if k < n_kx else hT[:, k - n_kx, :]

                ks = list(range(n_k)) if t > 0 else list(range(n_kx))
                G = [psum.tile([128, 512], F32, tag="ps", name=f"g_{m}_{t}_{n}")
                     for n in range(8)]
                for half in (0, 1):
                    for k in ks:
                        for n in range(half * 4, half * 4 + 4):
                            nc.tensor.matmul(
                                G[n][:], st(k), ww[:, k, n * 512:(n + 1) * 512],
                                start=(k == ks[0]), stop=(k == ks[-1]))

                # gate n-chunks: i: G[0:2], f: G[2:4], g: G[4:6], o: G[6:8]
                for j in (0, 1):
                    def gate_in(idx):
                        src = G[idx][:]
                        if has_gate_bias:
                            gs = work.tile([128, 512], F32, tag="gsb")
                            nc.vector.tensor_add(
                                gs[:], src, gb_sb[:, idx * 512:(idx + 1) * 512])
                            src = gs[:]
                        return src

                    sl = slice(j * 512, (j + 1) * 512)
                    sig_f = work.tile([128, 512], F32, tag="sig_f")
                    nc.scalar.activation(sig_f[:], gate_in(2 + j), AF.Sigmoid)
                    if t > 0:
                        nc.vector.tensor_mul(c_st[:, sl], sig_f[:], c_st[:, sl])
                    sig_i = work.tile([128, 512], F32, tag="sig_i")
                    nc.scalar.activation(sig_i[:], gate_in(0 + j), AF.Sigmoid)
                    tanh_g = work.tile([128, 512], F32, tag="tanh_g")
                    nc.scalar.activation(tanh_g[:], gate_in(4 + j), AF.Tanh)
                    if t > 0:
                        tmp2 = work.tile([128, 512], F32, tag="tmp2")
                        nc.vector.tensor_mul(tmp2[:], sig_i[:], tanh_g[:])
                        nc.vector.tensor_add(c_st[:, sl], c_st[:, sl], tmp2[:])
                    else:
                        nc.vector.tensor_mul(c_st[:, sl], sig_i[:], tanh_g[:])
                    tc_t = work.tile([128, 512], F32, tag="tc_t")
                    nc.scalar.activation(tc_t[:], c_st[:, sl], AF.Tanh)
                    sig_o = work.tile([128, 512], F32, tag="sig_o")
                    nc.scalar.activation(sig_o[:], gate_in(6 + j), AF.Sigmoid)
                    if t < T - 1:
                        nc.vector.tensor_mul(h_bf[:, sl], sig_o[:], tc_t[:])
                    else:
                        if m == "audio":
                            nc.vector.tensor_mul(fused_acc[:, sl], sig_o[:], tc_t[:])
                        else:
                            hf = work.tile([128, 512], F32, tag="hf")
                            nc.vector.tensor_mul(hf[:], sig_o[:], tc_t[:])
                            nc.vector.tensor_mul(
                                fused_acc[:, sl], fused_acc[:, sl], hf[:])
                if t < T - 1:
                    for k in range(NKH):
                        tp = tpsum.tile([128, 512], F32, tag="tp",
                                        name=f"tp_{m}_{t}_{k}")
                        tpv = tp[:, 0:128].bitcast(BF16)[:, 0:128]
                        nc.tensor.transpose(
                            tpv, h_bf[:, k * 128:(k + 1) * 128], ident_bf[:])
                        nc.vector.tensor_copy(hT[:, k, :], tpv)

        # ---------------- audio ----------------
        with tc.tile_pool(name="xta", bufs=1) as xtap, \
             tc.tile_pool(name="wwa", bufs=1) as wwa:
            xt_a = xtap.tile([128, T, 128], BF16)
            for t in range(T):
                nc.sync.dma_start_transpose(xt_a[:, t, :], x_d["audio"][t])
            recurrence("audio", lambda t, k: xt_a[:, t, :], 1, wwa)

        # ------------- resnet / c3d -------------
        RCH = 512  # rows per proj chunk (4 time steps)
        for m in ("resnet", "c3d"):
            dk = DIMS[m] // 128
            with (
                tc.tile_pool(name="wtp", bufs=1) as wtp,
                tc.tile_pool(name="xtp", bufs=2) as xtp,
                tc.tile_pool(name="evp", bufs=4) as evp,
            ):
                bp = const.tile([128, NKH], F32, tag=f"bp_{m}")
                nc.sync.dma_start(bp[:], bp_d[m].rearrange("(mo p) -> p mo", p=128))
                wt = wtp.tile([128, dk, P], BF16, tag="wt")
                wtr = wt_d[m].rearrange("(ko p) n -> p ko n", p=128)
                for k in range(dk):
                    nc.sync.dma_start(wt[:, k], wtr[:, k])
                for r in range(0, T * BS // RCH):
                    xt = xtp.tile([128, dk, RCH], BF16, tag="xt")
                    for tt in range(RCH // BS):
                        t = (r * RCH) // BS + tt
                        nc.sync.dma_start_transpose(
                            xt[:, :, tt * BS:(tt + 1) * BS], x_d[m][t])
                    for half in (0, 1):
                        pp = [psum.tile([128, 512], F32, tag="ps",
                                        name=f"pj_{m}_{r}_{half}_{mm}")
                              for mm in range(4)]
                        for k in range(dk):
                            for mm in range(4):
                                nc.tensor.matmul(
                                    pp[mm][:, 0:RCH],
                                    wt[:, k, (half * 4 + mm) * 128:
                                             (half * 4 + mm + 1) * 128],
                                    xt[:, k, :],
                                    start=(k == 0), stop=(k == dk - 1))
                        for mm in range(4):
                            mo = half * 4 + mm
                            ev = evp.tile([128, RCH], BF16, tag="ev")
                            nc.scalar.activation(
                                ev[:], pp[mm][:, 0:RCH], AF.Identity,
                                bias=bp[:, mo:mo + 1])
                            for tt in range(RCH // BS):
                                t = (r * RCH) // BS + tt
                                nc.sync.dma_start(
                                    pjt_d[m][t, mo],
                                    ev[:, tt * BS:(tt + 1) * BS])

            with tc.tile_pool(name="pjs", bufs=3) as pjs, \
                 tc.tile_pool(name="wwp", bufs=1) as wwp:
                pjt_tiles = {}

                def stream_pjt(t, k, m=m, pjs=pjs, pjt_tiles=pjt_tiles):
                    if t not in pjt_tiles:
                        pt = pjs.tile([128, NKH, BS], BF16, tag="pjt")
                        nc.sync.dma_start(
                            pt[:], pjt_d[m][t].rearrange("mo p b -> p mo b"))
                        pjt_tiles.clear()
                        pjt_tiles[t] = pt
                    return pjt_tiles[t][:, k, :]

                recurrence(m, stream_pjt, NKH, wwp)

        # ---------------- final ----------------
        with tc.tile_pool(name="fin", bufs=1) as fin:
            wo = fin.tile([128, NKH, NCLS], F32)
            nc.sync.dma_start(wo[:], wout_d.rearrange("(ko p) n -> p ko n", p=128))
            ops = psum.tile([128, 512], F32, tag="ps", name="out_ps")
            for k in range(NKH):
                tp = tpsum.tile([128, 512], F32, tag="tp", name=f"ft_{k}")
                nc.tensor.transpose(
                    tp[:, 0:128], fused_acc[:, k * 128:(k + 1) * 128], ident_f32[:])
                ft = work.tile([128, 128], F32, tag="ft")
                nc.vector.tensor_copy(ft[:], tp[:, 0:128])
                nc.tensor.matmul(ops[:, 0:NCLS], ft[:], wo[:, k, :],
                                 start=(k == 0), stop=(k == NKH - 1))
            osb = work.tile([128, NCLS], F32, tag="osb")
            nc.vector.tensor_copy(osb[:], ops[:, 0:NCLS])
            nc.sync.dma_start(out_d[:], osb[:])

    nc.compile()
    return nc


def _bf16(a):
    return np.ascontiguousarray(a).astype(ml_dtypes.bfloat16)


def host_prep(inputs):
    f = np.float32
    xs = {"audio": inputs["audio_features"], "resnet": inputs["resnet_features"],
          "c3d": inputs["c3d_features"]}
    xt = {m: np.swapaxes(np.asarray(v, f), 0, 1) for m, v in xs.items()}

    wt = {"resnet": _bf16(np.asarray(inputs["W_resnet"], f).T),
          "c3d": _bf16(np.asarray(inputs["W_c3d"], f).T)}
    bp = {"resnet": np.asarray(inputs["b_resnet"], f),
          "c3d": np.asarray(inputs["b_c3d"], f)}

    dirs = {}
    has_gate_bias = False
    for d in ("fwd", "rev"):
        ww = {}
        gb = {}
        for m in MODS:
            wih = np.asarray(inputs[f"{m}_{d}_Wih"], f)
            whh = np.asarray(inputs[f"{m}_{d}_Whh"], f)
            bih = np.asarray(inputs[f"{m}_{d}_bih"], f)
            bhh = np.asarray(inputs[f"{m}_{d}_bhh"], f)
            if m == "audio":
                wa = np.asarray(inputs["W_audio"], f)
                wcomb = wih @ wa                        # (4H, AUD)
                ww[m] = _bf16(np.concatenate([wcomb.T, whh.T], axis=0))
                gb[m] = (wih @ np.asarray(inputs["b_audio"], f) + bih + bhh).astype(f)
            else:
                ww[m] = _bf16(np.concatenate([wih.T, whh.T], axis=0))
                gb[m] = (bih + bhh).astype(f)
            if np.any(gb[m] != 0):
                has_gate_bias = True
        wout_half = (np.asarray(inputs["W_out"], f)[:, :H].T if d == "fwd"
                     else np.asarray(inputs["W_out"], f)[:, H:].T)
        dirs[d] = {"ww": ww, "gb": gb, "wout": np.ascontiguousarray(wout_half)}

    in_maps = []
    for core in range(8):
        d = "fwd" if core < 4 else "rev"
        s = core % 4
        rows = slice(s * BS, (s + 1) * BS)
        im = {}
        for m in MODS:
            xm = xt[m][:, rows]
            if d == "rev":
                xm = xm[::-1]
            im[f"x_{m}"] = _bf16(xm)
            im[f"ww_{m}"] = dirs[d]["ww"][m]
            im[f"gb_{m}"] = dirs[d]["gb"][m]
        for m in ("resnet", "c3d"):
            im[f"wt_{m}"] = wt[m]
            im[f"bp_{m}"] = bp[m]
        im["wout"] = dirs[d]["wout"]
        in_maps.append(im)
    return in_maps, has_gate_bias


def assemble(results, inputs):
    out = np.zeros((B, NCLS), np.float32)
    for s in range(4):
        rows = slice(s * BS, (s + 1) * BS)
        out[rows] = results[s]["out_partial"] + results[4 + s]["out_partial"]
    out += np.asarray(inputs["b_out"], np.float32)[None, :]
    return out


def kernel(**inputs):
    global LAST_RESULTS
    in_maps, has_gate_bias = host_prep(inputs)
    nc = build_program(has_gate_bias)
    res = bass_utils.run_bass_kernel_spmd(
        nc, in_maps, core_ids=list(range(8)), trace=TRACE)
    LAST_RESULTS = res
    return assemble(res.results, inputs)
